# revision 1
# baseline (speedup 1.0000x reference)
"""Cross-attention (B=8, C=128, x 64x64 tokens, y 32x32 tokens) on 8 TRN2 cores.

Strategy: data-parallel over batch B (1 batch element per NeuronCore).
Per core, everything is kept in "channels on partitions" layout so no
on-chip transposes are needed:

  xT = x[b] viewed as [C=128, N=4096]      (natural layout of NCHW)
  yT = y[b] viewed as [C=128, M=1024]
  KT[d, m] = (Wk @ yT)[d, m] + bk[d]          matmul, lhsT = Wk^T (host-prep)
  V[m, d]  = (yT^T @ Wv^T)[m, d] + bv[d]      matmul, lhsT = yT slice
  K2[c, m] = sum_d Wq[d, c] KT[d, m]          folds the Q projection into S^T:
  sb[m]    = sum_d KT[d, m] bq[d]               S^T = K2^T @ xT + sb[m]
  ST[m, n] = sum_c K2[c, m] xT[c, n]          scores^T, m on partitions
  PT[m, n] = exp(scale * ST[m, n] + scale*sb[m])   (activation bias, no max-sub)
  zT[d, n] = sum_m V[m, d] PT[m, n]           accumulated over m tiles in PSUM
  rs[:, n] = sum_m PT[m, n]                   ones-matmul (broadcast over parts)
  out[d,n] = xT[d, n] + zT[d, n] / rs[:, n]

The max-subtraction skip is safe here: scores*scale ~ N(0,1) (x,y ~ N(0,1),
W ~ N(0,1)/sqrt(C)), so exp() stays within ~e^10 of 1.0 -- far inside fp32
range -- and the result is mathematically identical to softmax.

All matmuls run float32r (1 col/cycle on the PE at free-dim >= 256, vs 4
cycles for float32; ~2.6e-4 scale-relative output error vs the fp32
reference). Walrus requires fp32r operands to be produced "rounded", so
DRAM inputs are declared float32r (DMA output counts) and on-chip operand
tiles are written as float32r by their DVE/ACT producer ops.
"""

import os

import numpy as np

import concourse.bass as bass  # noqa: F401  (bass types used via tile/bacc)
import concourse.mybir as mybir
import concourse.tile as tile
from concourse import bacc
from concourse.bass_utils import run_bass_kernel_spmd

B = 8
C = 128
N = 64 * 64   # 4096 x-tokens per batch element
M = 32 * 32   # 1024 y-tokens per batch element
NCHUNK = 512  # psum-bank sized n chunk
NCH = N // NCHUNK  # 8
MT = M // 128      # 8 m tiles
SCALE = 1.0 / float(np.sqrt(C))
FP = mybir.dt.float32
FPR = mybir.dt.float32r
EXPFN = mybir.ActivationFunctionType.Exp

# PE warmup: dummy fp32r matmuls issued before the input DMAs land, so the
# HAM clock gate reaches K=8/8 (2.4 GHz) before the real matmuls start and
# stays there (any >3.4us PE idle re-throttles to 1.2 GHz).
WARMUP_MMS = 20


def _build():
    nc = bacc.Bacc("TRN2", target_bir_lowering=False, debug=False, num_devices=B)

    x_d = nc.dram_tensor("x", [C, N], FPR, kind="ExternalInput").ap()
    y_d = nc.dram_tensor("y", [C, M], FPR, kind="ExternalInput").ap()
    # all weights+biases packed into one tensor: [wkT | wvT | wq | bq | bk]
    # -- one DMA with 1.5KB-per-partition descriptors instead of five DMAs
    # (the [C,1] bias loads alone were 128 4-byte descriptors each)
    wp_d = nc.dram_tensor("wpack", [C, 3 * C + 2], FPR, kind="ExternalInput").ap()
    bv_d = nc.dram_tensor("bv", [1, C], FP, kind="ExternalInput").ap()
    out_d = nc.dram_tensor("out", [C, N], FP, kind="ExternalOutput").ap()

    with tile.TileContext(nc) as tc:
        with (
            tc.tile_pool(name="const", bufs=1) as cpool,
            tc.tile_pool(name="work", bufs=4) as wpool,
            tc.tile_pool(name="ps_work", bufs=4, space="PSUM") as ps_work,
            tc.tile_pool(name="ps_zt", bufs=2, space="PSUM") as ps_zt,
            tc.tile_pool(name="ps_rs", bufs=1, space="PSUM") as ps_rs,
        ):
            xT = cpool.tile([C, N], FPR)
            yT = cpool.tile([C, M], FPR)
            wpack = cpool.tile([C, 3 * C + 2], FPR)
            wkT = wpack[:, 0:C]
            wvT = wpack[:, C : 2 * C]
            wq = wpack[:, 2 * C : 3 * C]
            bq = wpack[:, 3 * C : 3 * C + 1].bitcast(FP)
            bk = wpack[:, 3 * C + 1 : 3 * C + 2].bitcast(FP)
            bv_row = cpool.tile([1, C], FP)
            ones_col = cpool.tile([1, C], FP)
            bv_bcast = cpool.tile([C, C], FP)
            ones_sq = cpool.tile([C, C], FPR)
            KT = cpool.tile([C, M], FPR)
            K2 = cpool.tile([C, M], FPR)
            V = cpool.tile([C, M], FPR)  # slice mt: [m_local=128, d=128]
            sb = cpool.tile([C, MT], FP)  # scale*(KT^T bq) per m-tile, exp bias
            warm = cpool.tile([C, NCHUNK], FPR)

            ones_f = cpool.tile([C, NCHUNK], FP)
            nc.gpsimd.memset(ones_f[:], 1.0)
            nc.gpsimd.memset(ones_col[:], 1.0)
            nc.vector.tensor_copy(warm[:], ones_f[:])
            nc.vector.tensor_copy(ones_sq[:], ones_f[:, :C])

            # PE warmup burst: no data dependencies, keeps PE busy (and the
            # HAM clock warm) while the input DMAs land.
            for w in range(WARMUP_MMS):
                wps = ps_work.tile([C, NCHUNK], FP, name="wps", tag="pswork")
                nc.tensor.matmul(wps[:], warm[:, :C], warm[:], start=True, stop=True)

            # input DMAs across three parallel queues (sync-HWDGE,
            # scalar-HWDGE, gpsimd-SWDGE): weights on scalar, y halves on
            # sync+gpsimd, then x chunks split column-wise across
            # sync+gpsimd in consumption order.
            wsplit = 2 * C  # wkT|wvT on scalar; wq|bq|bk on sync
            nc.scalar.dma_start(wpack[:, :wsplit], wp_d[:, :wsplit])
            nc.sync.dma_start(wpack[:, wsplit:], wp_d[:, wsplit:])
            nc.gpsimd.dma_start(bv_row[:], bv_d[:])
            nc.gpsimd.dma_start(yT[:, : M // 2], y_d[:, : M // 2])
            nc.scalar.dma_start(yT[:, M // 2 :], y_d[:, M // 2 :])
            for c in range(4):
                a = slice(c * (N // 4), c * (N // 4) + N // 8)
                b = slice(c * (N // 4) + N // 8, (c + 1) * (N // 4))
                nc.sync.dma_start(xT[:, a], x_d[:, a])
                nc.gpsimd.dma_start(xT[:, b], x_d[:, b])

            # bv broadcast across partitions via a K=1 fp32 matmul
            bvps = ps_work.tile([C, C], FP, name="bvps", tag="pswork")
            nc.tensor.matmul(bvps[:], ones_col[:], bv_row[:], start=True, stop=True)
            nc.vector.tensor_copy(bv_bcast[:], bvps[:])

            # projections, emitted in dependency-ready order: everything
            # gated only on the first y half first, then the second half.
            def k_proj(j):
                sl = slice(j * NCHUNK, (j + 1) * NCHUNK)
                kps = ps_work.tile([C, NCHUNK], FP, name="kps", tag="pswork")
                nc.tensor.matmul(kps[:], wkT[:], yT[:, sl], start=True, stop=True)
                nc.vector.tensor_scalar_add(KT[:, sl], kps[:], bk[:])

            def v_proj(mt):
                msl = slice(mt * 128, (mt + 1) * 128)
                vps = ps_work.tile([C, C], FP, name="vps", tag="pswork")
                nc.tensor.matmul(vps[:], yT[:, msl], wvT[:], start=True, stop=True)
                nc.vector.tensor_add(V[:, msl], vps[:], bv_bcast[:])

            def k2_proj(j):
                # K2[c, m] = sum_d Wq[d, c] KT[d, m]  (folded Q projection)
                sl = slice(j * NCHUNK, (j + 1) * NCHUNK)
                k2ps = ps_work.tile([C, NCHUNK], FP, name="k2ps", tag="pswork")
                nc.tensor.matmul(k2ps[:], wq[:], KT[:, sl], start=True, stop=True)
                nc.vector.tensor_copy(K2[:, sl], k2ps[:])

            def sb_proj(mt):
                # sb[m-tile] = scale * sum_d KT[d, m] bq[d] -> exp bias cols
                msl = slice(mt * 128, (mt + 1) * 128)
                sbps = ps_work.tile([C, 1], FP, name="sbps", tag="pswork")
                nc.tensor.matmul(
                    sbps[:], KT[:, msl].bitcast(FP), bq[:], start=True, stop=True
                )
                nc.vector.tensor_scalar_mul(sb[:, mt : mt + 1], sbps[:], SCALE)

            def filler(k):
                # dependency-free matmuls that plug PE idle bubbles in the
                # projection phase (keeps the HAM clock-gate at 8/8)
                for _ in range(k):
                    fps = ps_work.tile(
                        [C, NCHUNK], FP, name="fps", tag="pswork"
                    )
                    nc.tensor.matmul(
                        fps[:], warm[:, :C], warm[:], start=True, stop=True
                    )

            k_proj(0)
            filler(2)
            for mt in range(MT // 2):
                v_proj(mt)
            filler(2)
            k2_proj(0)
            filler(2)
            for mt in range(MT // 2):
                sb_proj(mt)
            k_proj(1)
            filler(2)
            for mt in range(MT // 2, MT):
                v_proj(mt)
            filler(2)
            k2_proj(1)
            filler(2)
            for mt in range(MT // 2, MT):
                sb_proj(mt)

            # attention main loop: per 512-col n-chunk j, accumulate over
            # the 8 m-tiles. st bufs=4 gives the PE ~3 iterations of
            # lookahead so the exp latency stays off the critical path.
            opair_box = [None]
            for j in range(NCH):
                nsl = slice(j * NCHUNK, (j + 1) * NCHUNK)
                zt = ps_zt.tile([C, NCHUNK], FP, name="zt", tag="zt")
                rs = ps_rs.tile([C, NCHUNK], FP, name="rs", tag="rs", bufs=2)
                for mt in range(MT):
                    msl = slice(mt * 128, (mt + 1) * 128)
                    st = ps_work.tile([C, NCHUNK], FP, name="st", tag="pswork")
                    nc.tensor.matmul(
                        st[:], K2[:, msl], xT[:, nsl], start=True, stop=True
                    )
                    pt = wpool.tile([C, NCHUNK], FPR, name="pt", tag="pt", bufs=6)
                    nc.scalar.activation(
                        pt[:], st[:], EXPFN, bias=sb[:, mt : mt + 1], scale=SCALE
                    )
                    nc.tensor.matmul(
                        zt[:], V[:, msl], pt[:],
                        start=(mt == 0), stop=(mt == MT - 1),
                    )
                    nc.tensor.matmul(
                        rs[:], ones_sq[:], pt[:],
                        start=(mt == 0), stop=(mt == MT - 1),
                    )
                # epilogue: out = x + zt/rs, written into a pair buffer so
                # stores go out as [C, 1024] transfers (halved descriptor
                # count); the very last chunk is processed in quarters so the
                # tail-exposed final stores are short.
                if j % 2 == 0:
                    opair = wpool.tile(
                        [C, 2 * NCHUNK], FP, name="opair", tag="opair", bufs=2
                    )
                    opair_box[0] = opair
                opair = opair_box[0]
                half = opair[:, (j % 2) * NCHUNK : (j % 2 + 1) * NCHUNK]
                if j == NCH - 1:
                    for q in range(2):
                        qn = NCHUNK // 2
                        qs = slice(q * qn, (q + 1) * qn)
                        gq = slice(j * NCHUNK + q * qn, j * NCHUNK + (q + 1) * qn)
                        hq = half[:, qs]
                        recip = wpool.tile([C, qn], FP, name="recip", tag="recip")
                        nc.vector.reciprocal_approx_fast(recip[:], rs[:, qs])
                        nc.vector.tensor_mul(hq, zt[:, qs], recip[:])
                        nc.vector.tensor_add(hq, hq, xT[:, gq].bitcast(FP))
                        qeng = nc.sync if q == 0 else nc.scalar
                        qeng.dma_start(out_d[:, gq], hq)
                else:
                    recip = wpool.tile([C, NCHUNK], FP, name="recip", tag="recip")
                    nc.vector.reciprocal_approx_fast(recip[:], rs[:])
                    nc.vector.tensor_mul(half, zt[:], recip[:])
                    nc.vector.tensor_add(half, half, xT[:, nsl].bitcast(FP))
                    if j % 2 == 1:
                        peng = nc.sync if (j // 2) % 2 == 0 else nc.gpsimd
                        psl = slice((j - 1) * NCHUNK, (j + 1) * NCHUNK)
                        peng.dma_start(out_d[:, psl], opair[:])
                    elif j == NCH - 2:
                        nc.gpsimd.dma_start(out_d[:, nsl], half)

    nc.compile()
    return nc


_CACHE = {}


def _get_nc():
    if "nc" not in _CACHE:
        _CACHE["nc"] = _build()
    return _CACHE["nc"]


def _make_in_maps(inputs):
    x = np.ascontiguousarray(np.asarray(inputs["x"], np.float32)).reshape(B, C, N)
    y = np.ascontiguousarray(np.asarray(inputs["y"], np.float32)).reshape(B, C, M)
    wq = np.asarray(inputs["Wq"], np.float32)
    wkT = np.asarray(inputs["Wk"], np.float32).T
    wvT = np.asarray(inputs["Wv"], np.float32).T
    bq = np.asarray(inputs["bq"], np.float32).reshape(C, 1)
    bk = np.asarray(inputs["bk"], np.float32).reshape(C, 1)
    bv = np.ascontiguousarray(np.asarray(inputs["bv"], np.float32).reshape(1, C))
    wpack = np.ascontiguousarray(
        np.concatenate([wkT, wvT, wq, bq, bk], axis=1)
    )
    return [
        {
            "x": np.ascontiguousarray(x[b]),
            "y": np.ascontiguousarray(y[b]),
            "wpack": wpack,
            "bv": bv,
        }
        for b in range(B)
    ]


def _run(inputs, trace=False, **kwargs):
    nc = _get_nc()
    in_maps = _make_in_maps(inputs)
    last_err = None
    for attempt in range(3):
        try:
            res = run_bass_kernel_spmd(
                nc, in_maps, list(range(B)), trace=trace, **kwargs
            )
            break
        except Exception as e:  # transient NRT device wedge: retry
            last_err = e
            if attempt == 2:
                raise
            import time

            time.sleep(15)
    out = np.stack(
        [np.asarray(res.results[b]["out"], np.float32).reshape(C, 64, 64)
         for b in range(B)]
    )
    return out, res


def kernel(**inputs) -> np.ndarray:
    out, _ = _run(inputs, trace=False)
    return out


if __name__ == "__main__":
    # smoke: build only
    os.environ.setdefault("BASS_NEVER_TRACE", "")
    _get_nc()
    print("build ok")

